# revision 2
# baseline (speedup 1.0000x reference)
"""Gumbel top-k (sequential masking) Trainium2 kernel — v3.2 (index-based).

Problem: B=64 rows, N=16384, K=16 sequential top-1+mask steps.
  noisy = logits + gumbel; per step j: soft_j = softmax(noisy_masked/TAU),
  select argmax, mask it; outputs st (one-hot) and softs, each [K, B, N].

Strategy (data-parallel over batch, 8 rows/core on 8 cores; each row
occupies 16 SBUF partitions x 1024 columns, and each row maps 1:1 onto
one GPSIMD Q7 core):
  - Selection runs on z = noisy directly (order(z) == order(e)); per-half
    max8 + max_index give per-partition top-8 values AND column indices.
    A stream_shuffle gather makes every partition of a row hold all the
    row's candidates; max8/match_replace/max8 yield the row's top-16
    values and max_index their candidate slots. Winner columns are
    recovered per-partition by an expand/compare/reduce against the
    partition's own index table, then broadcast within each row by
    shuffle+max folds. All on DVE; gpsimd runs ONLY scatter_adds (a
    single ucode library -> no library-swap stalls).
  - e = exp(z/TAU) is bf16. The k-step masking is INDEX-based: two
    alternating chain tiles (even/odd planes) are updated in place by
    single-entry gpsimd scatter_add ops that add -e_bf at the winner's
    byte pair (exact cancellation to 0.0 in bf16; per-Q7-core wrapped
    index lists give per-row columns). No value matching anywhere.
  - soft_j = e_chain * (1/S_j): one DVE tensor_scalar mult per plane
    (bf16 -> DVE 2x mode), S_j from f32 exp-sum accum + prefix sums.
  - st planes are fp8(e4m3) BYTES packed in pairs inside u16 containers:
    one DVE tensor_scalar (f16-iota == paircol) * payload per plane,
    payload 0x0038/0x3800 places fp8(1.0) at the even/odd column.
    Host reinterprets bytes. 1 byte/elem of st traffic.
  - Output bytes/core: softs bf16 4 MiB + st fp8 2 MiB (vs 12 MiB f32+bf16).
"""

import numpy as np
from contextlib import ExitStack

import concourse.bacc as bacc
import concourse.bass as bass
import concourse.mybir as mybir
import concourse.tile as tile
from concourse.bass_utils import run_bass_kernel_spmd

F32 = mybir.dt.float32
F16 = mybir.dt.float16
BF16 = mybir.dt.bfloat16
I16 = mybir.dt.int16
U16 = mybir.dt.uint16
B, N, NCORES = 64, 16384, 8
R = B // NCORES          # rows per core = 8
QP = 16                  # partitions per row
FREE = N // QP           # 1024
H = FREE // 2            # 512
P = 128                  # SBUF partitions
INV_TAU = 1.5            # 1/(2/3), exact in fp32
ALU = mybir.AluOpType
AF = mybir.ActivationFunctionType
NEG_BIG = -3.0e38

_module_cache = {}


def _fold_mask(d):
    """stream_shuffle mask rotating by d within each 16-partition group."""
    return [(i & 16) | ((i + d) & 15) for i in range(32)]


def _build(K: int):
    nc = bacc.Bacc("TRN2", target_bir_lowering=False, debug=False,
                   num_devices=NCORES)
    z_d = nc.dram_tensor("z", [P, FREE], F32, kind="ExternalInput")
    pid_d = nc.dram_tensor("pid", [P, 1], F32, kind="ExternalInput")
    iot_d = nc.dram_tensor("iot", [P, H], F16, kind="ExternalInput")
    cidx_d = nc.dram_tensor("cidx", [P, 16], F32, kind="ExternalInput")
    softs_d = nc.dram_tensor("softs", [K, P, FREE], BF16, kind="ExternalOutput")
    st_d = nc.dram_tensor("st", [K, P, H], U16, kind="ExternalOutput")

    with tile.TileContext(nc) as tc, ExitStack() as ctx:
        io = ctx.enter_context(tc.tile_pool(name="io", bufs=1))
        sp = io
        op_s = ctx.enter_context(tc.tile_pool(name="out", bufs=8))
        op_h = op_s

        # ---- inputs (z halves first on both queues; consts behind) ----
        z = io.tile([P, FREE], F32, tag="in")
        nc.scalar.dma_start(out=z[:, 0:H], in_=z_d.ap()[:, 0:H])
        nc.sync.dma_start(out=z[:, H:FREE], in_=z_d.ap()[:, H:FREE])
        pidf = sp.tile([P, 1], F32, tag="pidf")      # partition id % 16
        iot = sp.tile([P, H], F16, tag="iot")        # 0..511 per partition
        cidx = sp.tile([P, 16], F32, tag="cidx")     # 0..15 per partition
        nc.sync.dma_start(out=pidf[:], in_=pid_d.ap())
        nc.sync.dma_start(out=cidx[:], in_=cidx_d.ap())
        nc.scalar.dma_start(out=iot[:], in_=iot_d.ap())

        # ---- e = exp(z/TAU) bf16, with f32 accumulated sums (halves) ----
        eA = io.tile([P, FREE], BF16, tag="eA")
        S0h = sp.tile([P, 2], F32, tag="S0h")
        nc.scalar.activation(eA[:, 0:H], z[:, 0:H], AF.Exp, scale=INV_TAU,
                             accum_out=S0h[:, 0:1])
        nc.scalar.activation(eA[:, H:FREE], z[:, H:FREE], AF.Exp,
                             scale=INV_TAU, accum_out=S0h[:, 1:2])
        eB = io.tile([P, FREE], BF16, tag="eB")
        nc.vector.tensor_copy(eB[:], eA[:])

        # ---- per-partition top-8 per half, values + indices ----
        # stage (f32-equivalent layout [P, 26]):
        #   [0:8] vals half0, [8:16] vals half1,
        #   u16 cols [32:40] idxs half0, [40:48] idxs half1 (each in [0,512))
        #   [24:26] exp-sum halves
        stage = sp.tile([P, 26], F32, tag="stage")
        stage_u16 = stage[:].bitcast(U16)
        nc.vector.max(stage[:, 0:8], z[:, 0:H])
        nc.vector.max(stage[:, 8:16], z[:, H:FREE])
        nc.vector.max_index(stage_u16[:, 32:40], stage[:, 0:8], z[:, 0:H])
        nc.vector.max_index(stage_u16[:, 40:48], stage[:, 8:16], z[:, H:FREE])
        nc.vector.tensor_copy(stage[:, 24:26], S0h[:])

        # ---- gather every partition's stage to all partitions of its row ----
        cand = sp.tile([P, QP * 26], F32, tag="cand")
        for k in range(QP):
            nc.vector.stream_shuffle(cand[:, 26 * k:26 * k + 26], stage[:],
                                     [k] * 16 + [16 + k] * 16)
        candv = cand[:].rearrange("p (q c) -> p q c", c=26)

        # flat copy of candidate values [P,256] f32
        cv = sp.tile([P, 256], F32, tag="cv")
        nc.vector.tensor_copy(cv[:].rearrange("p (q c) -> p q c", c=16),
                              candv[:, :, 0:16])

        # row exp-sum: reduce the 16 gathered per-partition accum pairs
        S0 = sp.tile([P, 1], F32, tag="S0")
        nc.vector.tensor_reduce(S0[:], candv[:, :, 24:26],
                                axis=mybir.AxisListType.XY, op=ALU.add)

        # ---- row top-16 (values, descending) + slots in candidate array ----
        vv = sp.tile([P, 16], F32, tag="vv")
        slots = sp.tile([P, 16], U16, tag="slots")
        c2 = sp.tile([P, 256], F32, tag="c2")
        nc.vector.max(vv[:, 0:8], cv[:])
        nc.vector.max_index(slots[:, 0:8], vv[:, 0:8], cv[:])
        nc.vector.match_replace(c2[:], vv[:, 0:8], cv[:], NEG_BIG)
        nc.vector.max(vv[:, 8:16], c2[:])
        nc.vector.max_index(slots[:, 8:16], vv[:, 8:16], c2[:])
        ev = sp.tile([P, 16], F32, tag="ev")
        nc.scalar.activation(ev[:], vv[:], AF.Exp, scale=INV_TAU)

        # ---- S_j prefix path emitted later; winner exps now (ACT) ----
        pf0 = sp.tile([P, 16], F32, tag="pf0")
        pf1 = sp.tile([P, 16], F32, tag="pf1")
        pf = [pf0, pf1]
        nc.vector.tensor_copy(pf[0][:], ev[:])
        cur = 0
        for sh in (1, 2, 4, 8):
            nxt = 1 - cur
            nc.vector.tensor_copy(pf[nxt][:, 0:sh], pf[cur][:, 0:sh])
            nc.vector.tensor_tensor(pf[nxt][:, sh:16], pf[cur][:, sh:16],
                                    pf[cur][:, 0:16 - sh], ALU.add)
            cur = nxt
        SSp = sp.tile([P, 16], F32, tag="SSp")
        nc.vector.tensor_scalar(SSp[:, 1:16], pf[cur][:, 0:15], -1.0, S0[:],
                                ALU.mult, ALU.add)
        nc.vector.tensor_copy(SSp[:, 0:1], S0[:])
        rj = sp.tile([P, 16], F32, tag="rj")
        nc.vector.reciprocal(rj[:], SSp[:])

        # ---- winner columns (owner-local), owner masks ----
        # w_i = slots & 15 (position in this partition's 16-entry idx table)
        # halfbit_i = (slots >> 3) & 1 ; owner q_i = slots >> 4
        wb = sp.tile([P, 16], U16, tag="wb")
        nc.vector.tensor_scalar(wb[:], slots[:], 15, None, ALU.bitwise_and)
        wf = sp.tile([P, 16], F32, tag="wf")
        nc.vector.tensor_copy(wf[:], wb[:])
        halfb = sp.tile([P, 16], U16, tag="halfb")
        nc.vector.tensor_scalar(halfb[:], slots[:], 3, None,
                                ALU.logical_shift_right)
        nc.vector.tensor_scalar(halfb[:], halfb[:], 1, None, ALU.bitwise_and)
        halff = sp.tile([P, 16], F32, tag="halff")
        nc.vector.tensor_copy(halff[:], halfb[:])
        Qt = sp.tile([P, 16], U16, tag="Qt")
        nc.vector.tensor_scalar(Qt[:], slots[:], 4, None,
                                ALU.logical_shift_right)
        Qf = sp.tile([P, 16], F32, tag="Qf")
        nc.vector.tensor_copy(Qf[:], Qt[:])
        own = sp.tile([P, 16], F32, tag="own")       # 1.0 where owner
        nc.vector.tensor_scalar(own[:], Qf[:], pidf[:], None, ALU.is_equal)

        # this partition's own 16 candidate columns as f32
        idx16f = sp.tile([P, 16], F32, tag="idx16f")
        nc.vector.tensor_copy(idx16f[:], stage_u16[:, 32:48])
        # M[p, i, w] = (wf_i == w);  colOwn_i = sum_w M * idx16f_w
        M = sp.tile([P, 256], F32, tag="M")
        Mv = M[:].rearrange("p (i w) -> p i w", w=16)
        nc.vector.tensor_tensor(
            Mv,
            wf[:].rearrange("p (i o) -> p i o", o=1).broadcast_to([P, 16, 16]),
            cidx[:].rearrange("p (o w) -> p o w", o=1).broadcast_to([P, 16, 16]),
            ALU.is_equal)
        nc.vector.tensor_tensor(
            Mv, Mv,
            idx16f[:].rearrange("p (o w) -> p o w", o=1).broadcast_to([P, 16, 16]),
            ALU.mult)
        colOwn = sp.tile([P, 16], F32, tag="colOwn")
        nc.vector.tensor_reduce(colOwn[:], Mv, axis=mybir.AxisListType.X,
                                op=ALU.add)
        # full column (0..1023) = colOwn + 512*halfbit; valid at owner only
        nc.vector.tensor_scalar(halff[:], halff[:], 512.0, None, ALU.mult)
        nc.vector.tensor_tensor(colOwn[:], colOwn[:], halff[:], ALU.add)

        # par (column parity), owner-masked pair column T2f (else -1)
        parb = sp.tile([P, 16], U16, tag="parb")
        nc.vector.tensor_copy(parb[:], colOwn[:])
        nc.vector.tensor_scalar(parb[:], parb[:], 1, None, ALU.bitwise_and)
        parf = sp.tile([P, 16], F32, tag="parf")
        nc.vector.tensor_copy(parf[:], parb[:])
        # T = (col+1)*own - 1 ; T2f = (T - par*own + own - 1)/2
        T = sp.tile([P, 16], F32, tag="T")
        nc.vector.tensor_scalar(T[:], colOwn[:], 1.0, None, ALU.add)
        nc.vector.tensor_tensor(T[:], T[:], own[:], ALU.mult)
        nc.vector.tensor_scalar(T[:], T[:], 1.0, None, ALU.subtract)
        parO = sp.tile([P, 16], F32, tag="parO")
        nc.vector.tensor_tensor(parO[:], parf[:], own[:], ALU.mult)
        T2f = sp.tile([P, 16], F32, tag="T2f")
        nc.vector.tensor_tensor(T2f[:], T[:], parO[:], ALU.subtract)
        nc.vector.tensor_tensor(T2f[:], T2f[:], own[:], ALU.add)
        nc.vector.tensor_scalar(T2f[:], T2f[:], 1.0, 0.5, ALU.subtract,
                                ALU.mult)

        # broadcast pair column to the whole row: max-fold of T2f (-1 elsewhere)
        T2bc = sp.tile([P, 16], F32, tag="T2bc")
        tmpf = sp.tile([P, 16], F32, tag="tmpf")
        nc.vector.tensor_copy(T2bc[:], T2f[:])
        for dd in (1, 2, 4, 8):
            nc.vector.stream_shuffle(tmpf[:], T2bc[:], _fold_mask(dd))
            nc.vector.tensor_tensor(T2bc[:], T2bc[:], tmpf[:], ALU.max)

        # per-plane owner-masked add pairs (f32): A{0,1}f[p, j] is the value
        # to add at byte 0/1 of the winner's pair (nonzero only at owner)
        t1 = sp.tile([P, 16], F32, tag="t1")
        nc.vector.tensor_tensor(t1[:], ev[:], own[:], ALU.mult)
        nc.vector.tensor_scalar(t1[:], t1[:], -1.0, None, ALU.mult)
        par0 = sp.tile([P, 16], F32, tag="par0")
        nc.vector.tensor_scalar(par0[:], parf[:], -1.0, 1.0, ALU.mult, ALU.add)
        A0f = sp.tile([P, 16], F32, tag="A0f")
        nc.vector.tensor_tensor(A0f[:], t1[:], par0[:], ALU.mult)
        A1f = sp.tile([P, 16], F32, tag="A1f")
        nc.vector.tensor_tensor(A1f[:], t1[:], parf[:], ALU.mult)

        # merged 2-entry scatter_add tables. Step s applies {D_s, D_{s+1}};
        # if both land in the same byte pair (T2bc equal), entry 0 carries
        # both adds and entry 1 is disabled (idx -1).
        pid0 = sp.tile([P, 1], F32, tag="pid0")
        nc.vector.tensor_scalar(pid0[:], pidf[:], 0.0, None, ALU.is_equal)
        pid1 = sp.tile([P, 1], F32, tag="pid1")
        nc.vector.tensor_scalar(pid1[:], pidf[:], 1.0, None, ALU.is_equal)
        mrg = sp.tile([P, 15], F32, tag="mrg")
        nc.vector.tensor_tensor(mrg[:], T2bc[:, 1:16], T2bc[:, 0:15],
                                ALU.is_equal)
        u1 = sp.tile([P, 16], F32, tag="u1")
        nc.vector.tensor_scalar(u1[:], T2bc[:], 1.0, None, ALU.add)
        nmrg = sp.tile([P, 15], F32, tag="nmrg")
        nc.vector.tensor_scalar(nmrg[:], mrg[:], -1.0, 1.0, ALU.mult, ALU.add)
        ia = sp.tile([P, 15], F32, tag="ia")
        nc.vector.tensor_scalar(ia[:], u1[:, 0:15], pid0[:], None, ALU.mult)
        ib = sp.tile([P, 15], F32, tag="ib")
        nc.vector.tensor_tensor(ib[:], u1[:, 1:16], nmrg[:], ALU.mult)
        nc.vector.tensor_scalar(ib[:], ib[:], pid1[:], None, ALU.mult)
        nc.vector.tensor_tensor(ia[:], ia[:], ib[:], ALU.add)
        nc.vector.tensor_scalar(ia[:], ia[:], 1.0, None, ALU.subtract)
        IDXP = sp.tile([P, 15], I16, tag="IDXP")
        nc.vector.tensor_copy(IDXP[:], ia[:])
        # A2 flat [P, 88] bf16: step s entries at [4s..4s+3] =
        #   [E0b0, E0b1, A0f[s+1], A1f[s+1]]
        A2 = sp.tile([P, 88], BF16, tag="A2")
        nc.vector.memset(A2[:], 0.0)
        A2v = A2[:].rearrange("p (s f) -> p s f", f=4)
        e0b0 = sp.tile([P, 15], F32, tag="e0b0")
        nc.vector.tensor_tensor(e0b0[:], mrg[:], A0f[:, 1:16], ALU.mult)
        nc.vector.tensor_tensor(e0b0[:], e0b0[:], A0f[:, 0:15], ALU.add)
        e0b1 = sp.tile([P, 15], F32, tag="e0b1")
        nc.vector.tensor_tensor(e0b1[:], mrg[:], A1f[:, 1:16], ALU.mult)
        nc.vector.tensor_tensor(e0b1[:], e0b1[:], A1f[:, 0:15], ALU.add)
        nc.vector.tensor_copy(A2v[:, 0:15, 0:1], e0b0[:])
        nc.vector.tensor_copy(A2v[:, 0:15, 1:2], e0b1[:])
        nc.vector.tensor_copy(A2v[:, 0:15, 2:3], A0f[:, 1:16])
        nc.vector.tensor_copy(A2v[:, 0:15, 3:4], A1f[:, 1:16])
        # init tables for the single-entry scatter_add taking eB to state 1
        Ainit = sp.tile([P, 32], BF16, tag="Ainit")
        nc.vector.memset(Ainit[:], 0.0)
        nc.vector.tensor_copy(Ainit[:, 0:1], A0f[:, 0:1])
        nc.vector.tensor_copy(Ainit[:, 1:2], A1f[:, 0:1])
        i1 = sp.tile([P, 1], F32, tag="i1")
        nc.vector.tensor_scalar(i1[:], u1[:, 0:1], pid0[:], None, ALU.mult)
        nc.vector.tensor_scalar(i1[:], i1[:], 1.0, None, ALU.subtract)
        IDX1 = sp.tile([P, 1], I16, tag="IDX1")
        nc.vector.tensor_copy(IDX1[:], i1[:])

        # st payload Df = 56 + par*14280 (0x0038 / 0x3800)
        Df = sp.tile([P, 16], F32, tag="Df")
        nc.vector.tensor_scalar(Df[:], parf[:], 14280.0, 56.0, ALU.mult,
                                ALU.add)

        def sa_init(t):
            nc.gpsimd.scatter_add(t[:].rearrange("p (n d) -> p n d", d=2),
                                  IDX1[:], Ainit[:].rearrange(
                                      "p (n d) -> p n d", d=2),
                                  channels=P, num_elems=H, d=2, num_idxs=16)

        def sa_step(t, s):
            nc.gpsimd.scatter_add(t[:].rearrange("p (n d) -> p n d", d=2),
                                  IDXP[:, s:s + 1],
                                  A2[:, 4 * s:4 * s + 32].rearrange(
                                      "p (n d) -> p n d", d=2),
                                  channels=P, num_elems=H, d=2, num_idxs=16)

        def produce(t, j):
            soft = op_s.tile([P, FREE], BF16, tag="soft")
            nc.vector.tensor_scalar(soft[:], t[:], rj[:, j:j + 1], None,
                                    ALU.mult)
            nc.sync.dma_start(out=softs_d.ap()[j], in_=soft[:])

        def emit_st(j):
            s = op_h.tile([P, H], U16, tag="st")
            nc.vector.tensor_scalar(s[:], iot[:], T2f[:, j:j + 1],
                                    Df[:, j:j + 1], ALU.is_equal, ALU.mult)
            nc.scalar.dma_start(out=st_d.ap()[j], in_=s[:])

        # ---- stream: even planes from eA, odd planes from eB ----
        if K == 1:
            produce(eA, 0)
            emit_st(0)
        else:
            sa_init(eB)                    # eB -> state 1
            produce(eA, 0)
            emit_st(0)
            for t in range(1, K):
                chain = eA if t % 2 == 0 else eB
                if t >= 2:
                    sa_step(chain, t - 2)  # state t-2 -> t via {D_t-2, D_t-1}
                produce(chain, t)
                emit_st(t)
    nc.compile()
    return nc


def kernel(logits, gumbel, k, trace=False):
    K = int(k)
    logits = np.ascontiguousarray(logits, dtype=np.float32)
    gumbel = np.ascontiguousarray(gumbel, dtype=np.float32)
    if K == 0:
        empty = np.zeros((0, B, N), dtype=np.float32)
        return empty, empty.copy()
    assert 1 <= K <= 16, f"unsupported k={K}"
    assert logits.shape == (B, N) and gumbel.shape == (B, N)

    if K not in _module_cache:
        _module_cache[K] = _build(K)
    nc = _module_cache[K]

    import ml_dtypes
    z_full = logits + gumbel
    pid = (np.arange(P, dtype=np.float32) % QP).reshape(P, 1)
    iot = np.broadcast_to(np.arange(H, dtype=np.float32), (P, H)).astype(
        ml_dtypes.float16 if hasattr(ml_dtypes, "float16") else np.float16)
    cidx = np.broadcast_to(np.arange(16, dtype=np.float32), (P, 16)).copy()
    iot = np.ascontiguousarray(iot)
    in_maps = []
    for c in range(NCORES):
        sl = slice(c * R, (c + 1) * R)
        in_maps.append({"z": z_full[sl].reshape(P, FREE), "pid": pid,
                        "iot": iot, "cidx": cidx})

    res = run_bass_kernel_spmd(nc, in_maps, core_ids=list(range(NCORES)),
                               trace=trace)

    st = np.empty((K, B, N), dtype=np.float32)
    softs = np.empty((K, B, N), dtype=np.float32)
    for c in range(NCORES):
        sl = slice(c * R, (c + 1) * R)
        softs[:, sl, :] = np.asarray(res.results[c]["softs"],
                                     dtype=np.float32).reshape(K, R, N)
        stb = np.ascontiguousarray(res.results[c]["st"])
        stbytes = stb.view(np.uint8).reshape(K, R, N)
        st[:, sl, :] = (stbytes == 0x38).astype(np.float32)

    global run_res
    run_res = res
    if trace:
        kernel.last_exec_time_ns = res.exec_time_ns
        kernel.last_results = res
    return st, softs
